# revision 5
# baseline (speedup 1.0000x reference)
"""Segment mean-pool (AspireConSent) Trainium2 kernel.

Computes, for hidden [B=64, S=512, D=768] f32 and sent_ids [B, S] int32 in
[0, 20]:
  doc_cls_reps = hidden[:, 0, :]                                  [B, D]
  sent_reps[b, m] = mean over tokens s with sent_ids[b, s] == m   [B, 20, D]
with empty-sentence means = 0 (count clamped to >= 1).

Strategy: data parallel over 8 NeuronCores (8 examples per core). On each
core the segment-sum is a one-hot matmul on the TensorEngine:
  O[s, m] = (sent_ids[s] == m)  ->  sums = O.T @ H,  counts = O.T @ ones.
"""

import sys

import numpy as np

for _p in ("/opt/trn_rl_repo", "/root/.axon_site/_ro/trn_rl_repo"):
    if _p not in sys.path:
        sys.path.append(_p)

import concourse.bass as bass
import concourse.bacc as bacc
import concourse.mybir as mybir
from concourse.bass_utils import run_bass_kernel_spmd
from concourse.tile import TileContext

N_CORES = 8
B, S, D = 64, 512, 768
BL = B // N_CORES  # examples per core
MS = 20            # real sentence buckets
M1 = MS + 1        # + the "no sentence" bucket
P = 128
NCH = S // P       # 128-token chunks per example

_CACHE = {}


def build_nc() -> bass.Bass:
    f32 = mybir.dt.float32
    i32 = mybir.dt.int32

    nc = bacc.Bacc()
    hidden = nc.declare_dram_parameter("hidden", [BL, S, D], f32, isOutput=False)
    sent_ids = nc.declare_dram_parameter("sent_ids", [BL, S], i32, isOutput=False)
    out_cls = nc.declare_dram_parameter("out_cls", [BL, D], f32, isOutput=True)
    out_sent = nc.declare_dram_parameter("out_sent", [BL, MS, D], f32, isOutput=True)

    with TileContext(nc) as tc:
        with (
            tc.tile_pool(name="const", bufs=1) as cpool,
            tc.tile_pool(name="h", bufs=3) as hpool,
            tc.tile_pool(name="outp", bufs=2) as opool,
            tc.tile_pool(name="small", bufs=2) as spool,
            tc.tile_pool(name="psum_s", bufs=2, space="PSUM") as pspool,
            tc.tile_pool(name="psum_c", bufs=2, space="PSUM") as pcpool,
        ):
            # ---- one-time setup: one-hot matrices for all 32 chunks ----
            iota_i = cpool.tile([P, M1], i32)
            nc.gpsimd.iota(iota_i[:], pattern=[[1, M1]], base=0, channel_multiplier=0)
            iota_f = cpool.tile([P, M1], f32)
            nc.vector.tensor_copy(iota_f[:], iota_i[:])

            ids_i = cpool.tile([BL * NCH, P], i32)  # [32, 128]
            nc.sync.dma_start(out=ids_i[:], in_=sent_ids.rearrange("b (c p) -> (b c) p", p=P))
            ids_f = cpool.tile([BL * NCH, P], f32)
            nc.vector.tensor_copy(ids_f[:], ids_i[:])
            # transpose [32, 128] -> [128, 32] via 32x32 DVE blocks
            ids_t = cpool.tile([P, BL * NCH], f32)
            for j in range(P // 32):
                nc.vector.transpose(out=ids_t[32 * j:32 * (j + 1), :], in_=ids_f[:, 32 * j:32 * (j + 1)])

            onehot = cpool.tile([P, BL * NCH * M1], f32)
            for k in range(BL * NCH):
                nc.vector.tensor_scalar(
                    out=onehot[:, k * M1:(k + 1) * M1],
                    in0=iota_f[:],
                    scalar1=ids_t[:, k:k + 1],
                    scalar2=None,
                    op0=mybir.AluOpType.is_equal,
                )
            ones = cpool.tile([P, 1], f32)
            nc.vector.memset(ones[:], 1.0)

            # ---- per-example stream ----
            for b in range(BL):
                ht = hpool.tile([P, NCH, D], f32)
                nc.sync.dma_start(out=ht[:], in_=hidden[b].rearrange("(c p) d -> p c d", p=P))

                psum_s = pspool.tile([M1, D], f32)
                psum_c = pcpool.tile([M1, 1], f32)

                def oh(c):
                    k = b * NCH + c
                    return onehot[:, k * M1:(k + 1) * M1]

                for c in range(NCH):
                    nc.tensor.matmul(psum_s[:, 0:512], oh(c), ht[:, c, 0:512],
                                     start=(c == 0), stop=(c == NCH - 1))
                for c in range(NCH):
                    nc.tensor.matmul(psum_s[:, 512:768], oh(c), ht[:, c, 512:768],
                                     start=(c == 0), stop=(c == NCH - 1))
                for c in range(NCH):
                    nc.tensor.matmul(psum_c[:, :], oh(c), ones[:],
                                     start=(c == 0), stop=(c == NCH - 1))

                # copy PSUM -> SBUF first (copies absorb the cross-engine
                # waits; the TensorScalar ISA struct has too few wait slots)
                cnt = spool.tile([M1, 1], f32)
                nc.vector.tensor_copy(cnt[:], psum_c[:])
                nc.vector.tensor_scalar_max(cnt[:], cnt[:], 1.0)
                recip = spool.tile([M1, 1], f32)
                nc.vector.reciprocal(recip[:], cnt[:])

                sums_sb = opool.tile([M1, D], f32)
                nc.scalar.copy(sums_sb[:], psum_s[:])
                outt = opool.tile([MS, D], f32)
                nc.vector.tensor_scalar_mul(outt[:], sums_sb[0:MS, :], recip[0:MS, :])
                nc.sync.dma_start(out=out_sent[b], in_=outt[:])
                nc.sync.dma_start(out=out_cls[b], in_=hidden[b, 0, :])
    nc.finalize()
    return nc


def _get_nc() -> bass.Bass:
    if "nc" not in _CACHE:
        _CACHE["nc"] = build_nc()
    return _CACHE["nc"]


def run(hidden, sent_ids, **spmd_kwargs):
    """Shard, run on 8 cores, gather. Returns (results_obj, cls, sent)."""
    hidden = np.ascontiguousarray(hidden, dtype=np.float32)
    sent_ids = np.ascontiguousarray(sent_ids, dtype=np.int32)
    nc = _get_nc()
    in_maps = [
        {
            "hidden": hidden[i * BL:(i + 1) * BL],
            "sent_ids": sent_ids[i * BL:(i + 1) * BL],
        }
        for i in range(N_CORES)
    ]
    res = run_bass_kernel_spmd(nc, in_maps, core_ids=list(range(N_CORES)), **spmd_kwargs)
    cls = np.concatenate([res.results[i]["out_cls"] for i in range(N_CORES)], axis=0)
    sent = np.concatenate([res.results[i]["out_sent"] for i in range(N_CORES)], axis=0)
    return res, cls, sent


def kernel(hidden, sent_ids, max_sents=20):
    assert int(max_sents) == MS
    _, cls, sent = run(hidden, sent_ids)
    return cls.astype(np.float32), sent.astype(np.float32)


if __name__ == "__main__":
    rng = np.random.default_rng(0)
    h = rng.standard_normal((B, S, D), dtype=np.float32)
    ids = rng.integers(0, M1, size=(B, S)).astype(np.int32)
    cls, sent = kernel(h, ids, MS)
    print("cls", cls.shape, "sent", sent.shape)


# revision 8
# speedup vs baseline: 1.3446x; 1.3446x over previous
"""Segment mean-pool (AspireConSent) Trainium2 kernel.

Computes, for hidden [B=64, S=512, D=768] f32 and sent_ids [B, S] int32 in
[0, 20]:
  doc_cls_reps = hidden[:, 0, :]                                  [B, D]
  sent_reps[b, m] = mean over tokens s with sent_ids[b, s] == m   [B, 20, D]
with empty-sentence means = 0 (count clamped to >= 1).

Strategy: data parallel over 8 NeuronCores (8 examples per core). On each
core the segment-sum is a one-hot matmul on the TensorEngine:
  O[s, m] = (sent_ids[s] == m)  ->  sums = O.T @ H,  counts = O.T @ ones.
"""

import sys

import numpy as np

for _p in ("/opt/trn_rl_repo", "/root/.axon_site/_ro/trn_rl_repo"):
    if _p not in sys.path:
        sys.path.append(_p)

import concourse.bass as bass
import concourse.bacc as bacc
import concourse.mybir as mybir
from concourse.bass_utils import run_bass_kernel_spmd
from concourse.tile import TileContext

N_CORES = 8
B, S, D = 64, 512, 768
BL = B // N_CORES  # examples per core
MS = 20            # real sentence buckets
M1 = MS + 1        # + the "no sentence" bucket
P = 128
NCH = S // P       # 128-token chunks per example

_CACHE = {}


def build_nc() -> bass.Bass:
    f32 = mybir.dt.float32
    i32 = mybir.dt.int32

    nc = bacc.Bacc()
    hidden = nc.declare_dram_parameter("hidden", [BL, S, D], f32, isOutput=False)
    sent_ids = nc.declare_dram_parameter("sent_ids", [BL, S], i32, isOutput=False)
    out_cls = nc.declare_dram_parameter("out_cls", [BL, D], f32, isOutput=True)
    out_sent = nc.declare_dram_parameter("out_sent", [BL, MS, D], f32, isOutput=True)

    bf16 = mybir.dt.bfloat16
    DP = D + 1  # 768 hidden cols + 1 ones column (for counts)

    with TileContext(nc) as tc:
        with (
            tc.tile_pool(name="const", bufs=1) as cpool,
            tc.tile_pool(name="h", bufs=4) as hpool,
            tc.tile_pool(name="outp", bufs=3) as opool,
            tc.tile_pool(name="small", bufs=3) as spool,
            tc.tile_pool(name="psum_s", bufs=2, space="PSUM") as pspool,
        ):
            # ---- one-time setup: one-hot matrices for all 32 chunks ----
            iota_i = cpool.tile([P, M1], i32)
            nc.gpsimd.iota(iota_i[:], pattern=[[1, M1]], base=0, channel_multiplier=0)
            iota_f = cpool.tile([P, M1], f32)
            nc.vector.tensor_copy(iota_f[:], iota_i[:])

            ids_i = cpool.tile([BL * NCH, P], i32)  # [32, 128]
            nc.sync.dma_start(out=ids_i[:], in_=sent_ids.rearrange("b (c p) -> (b c) p", p=P))
            ids_f = cpool.tile([BL * NCH, P], f32)
            nc.vector.tensor_copy(ids_f[:], ids_i[:])
            # transpose [32, 128] -> [128, 32] via 32x32 DVE blocks
            ids_t = cpool.tile([P, BL * NCH], f32)
            for j in range(P // 32):
                nc.vector.transpose(out=ids_t[32 * j:32 * (j + 1), :], in_=ids_f[:, 32 * j:32 * (j + 1)])

            # onehot[p, k, m] = (ids_t[p, k] == m), all 32 chunks in one op
            onehot = cpool.tile([P, BL * NCH, M1], bf16)
            nc.vector.tensor_tensor(
                out=onehot[:],
                in0=iota_f[:, None, :].to_broadcast([P, BL * NCH, M1]),
                in1=ids_t[:, :, None].to_broadcast([P, BL * NCH, M1]),
                op=mybir.AluOpType.is_equal,
            )

            # ---- per-example stream ----
            for b in range(BL):
                ht = hpool.tile([P, NCH, DP], bf16)
                # SWDGE DMA casts f32 -> bf16 inline (HBM reads stay f32)
                nc.gpsimd.dma_start(out=ht[:, :, 0:D], in_=hidden[b].rearrange("(c p) d -> p c d", p=P))
                nc.vector.memset(ht[:, :, D:DP], 1.0)

                psum_s = pspool.tile([M1, DP], f32)

                def oh(c):
                    return onehot[:, b * NCH + c, :]

                for c in range(NCH):
                    nc.tensor.matmul(psum_s[:, 0:512], oh(c), ht[:, c, 0:512],
                                     start=(c == 0), stop=(c == NCH - 1))
                for c in range(NCH):
                    nc.tensor.matmul(psum_s[:, 512:DP], oh(c), ht[:, c, 512:DP],
                                     start=(c == 0), stop=(c == NCH - 1))

                # counts live in psum_s[:, 768]; mean = sums * (1/max(cnt,1))
                cnt = spool.tile([M1, 1], f32)
                nc.vector.tensor_copy(cnt[:], psum_s[:, D:DP])
                nc.vector.tensor_scalar_max(cnt[:], cnt[:], 1.0)
                recip = spool.tile([M1, 1], f32)
                nc.vector.reciprocal(recip[:], cnt[:])

                outt = opool.tile([MS, D], f32)
                nc.scalar.activation(
                    out=outt[:],
                    in_=psum_s[0:MS, 0:D],
                    func=mybir.ActivationFunctionType.Copy,
                    scale=recip[0:MS, :],
                )
                nc.scalar.dma_start(out=out_sent[b], in_=outt[:])
                nc.scalar.dma_start(out=out_cls[b], in_=hidden[b, 0, :])
    nc.finalize()
    return nc


def _get_nc() -> bass.Bass:
    if "nc" not in _CACHE:
        _CACHE["nc"] = build_nc()
    return _CACHE["nc"]


def run(hidden, sent_ids, **spmd_kwargs):
    """Shard, run on 8 cores, gather. Returns (results_obj, cls, sent)."""
    hidden = np.ascontiguousarray(hidden, dtype=np.float32)
    sent_ids = np.ascontiguousarray(sent_ids, dtype=np.int32)
    nc = _get_nc()
    in_maps = [
        {
            "hidden": hidden[i * BL:(i + 1) * BL],
            "sent_ids": sent_ids[i * BL:(i + 1) * BL],
        }
        for i in range(N_CORES)
    ]
    res = run_bass_kernel_spmd(nc, in_maps, core_ids=list(range(N_CORES)), **spmd_kwargs)
    cls = np.concatenate([res.results[i]["out_cls"] for i in range(N_CORES)], axis=0)
    sent = np.concatenate([res.results[i]["out_sent"] for i in range(N_CORES)], axis=0)
    return res, cls, sent


def kernel(hidden, sent_ids, max_sents=20):
    assert int(max_sents) == MS
    _, cls, sent = run(hidden, sent_ids)
    return cls.astype(np.float32), sent.astype(np.float32)


if __name__ == "__main__":
    rng = np.random.default_rng(0)
    h = rng.standard_normal((B, S, D), dtype=np.float32)
    ids = rng.integers(0, M1, size=(B, S)).astype(np.int32)
    cls, sent = kernel(h, ids, MS)
    print("cls", cls.shape, "sent", sent.shape)
